# revision 29
# baseline (speedup 1.0000x reference)
"""Trainium2 Bass kernel for the ragged text-CNN problem (v2).

Math: conv[b,h,t] = w0_h . e_{t,b} + w1_h . e_{t+1,b} + cb_h over valid t,
scores = (masked max_t conv) @ out_w.T + out_b, e = concat(E[tok], U[tok]).

Fused table T[v, 0:64] = e_v . w0, T[v, 64:128] = e_v . w1 (bf16), so
conv[b,h,t] = T[tok_t, h] + T[tok_{t+1}, 64+h].  PAD rows of T carry -1e30
on the tap-0 half, making the ragged mask free.

Distribution (8 cores, pair-shared HBM on (2k, 2k+1)):
- Table rows padded to V'=51200, stored pair-interleaved in a pair-shared
  DRAM tensor t_full [25600 pairs, 256] bf16.  Half-A (rows [0,25600)) is
  written by the even member, half-B by the odd member.
- Each member builds ALPHA=16384 rows of its half locally, plus a
  PIECE=2304-row shard of the remaining 9216 rows; two concurrent 4-core
  AllGathers (evens / odds) exchange the shards, then a DRAM->DRAM copy
  lands them in t_full.  A 2-core barrier collective orders the partner's
  writes before the gather.
- Phase B: one transposed dma_gather per position (512B pair fetch,
  idx = tok>>1 int16), parity select via copy_predicated, tap-1 shift via
  a PE partition-extract matmul, per-sentence reduce_max with
  slot-uniform compile-time ranges (host balances sentences by length).
"""

import numpy as np

try:
    import concourse.bass as bass
except ImportError:  # harness runs from a bare directory
    import sys

    sys.path.insert(0, "/opt/trn_rl_repo")
    import concourse.bass as bass

import concourse.mybir as mybir
from concourse.bacc import Bacc
import concourse.tile as tile
from concourse.bass_utils import run_bass_kernel_spmd

V, D, H, S, B = 50000, 300, 64, 512, 256
NCORES = 8
BS = B // NCORES            # sentences per core (32)
F = 2 * H                   # fused feature width (128)
KD = 2 * D                  # contraction size (600)
NEG = -1.0e30
P = 128

VPAD = 51200                # padded vocab (rows)
HALF = VPAD // 2            # rows per half (25600)
ALPHA = 20480               # locally-built rows per half
CC = HALF - ALPHA           # collective-delivered rows per half (5120)
NPIECE = CC // 4            # rows per core's collective shard (1280)
NPAIR = VPAD // 2           # pair-rows in t_full (25600)
TROW = 256                  # elems per pair-row (bf16) = 512B

ATILES = ALPHA // 256       # 64 tiles of 256 rows
PTILES = NPIECE // 256      # 9 tiles
CHK = 5                     # contraction chunks of 120 rows (5*120=600)
CROW = 120

F32 = mybir.dt.float32
BF16 = mybir.dt.bfloat16
I16 = mybir.dt.int16
I32 = mybir.dt.int32


def build_nc(C, ranges):
    """Per-core SPMD program.  C = gather positions (mult of 1024);
    ranges = 32 compile-time (start, end) column ranges, slot-uniform."""
    Cc = C // 2             # gather chunk (mult of 512)
    NQ = C // 512           # 512-col pipeline steps

    nc = Bacc()
    eu_alpha = nc.dram_tensor("eu_alpha", [KD, ALPHA], BF16, kind="ExternalInput")
    eu_piece = nc.dram_tensor("eu_piece", [KD, NPIECE], BF16, kind="ExternalInput")
    w2 = nc.dram_tensor("w2", [KD, F], BF16, kind="ExternalInput")
    patch = nc.dram_tensor("patch", [1, TROW], F32, kind="ExternalInput")
    par = nc.dram_tensor("par", [1, 1], I32, kind="ExternalInput")
    idx_in = nc.dram_tensor("idx_in", [32, C // 16], I16, kind="ExternalInput")
    mv_in = nc.dram_tensor("mv_in", [P, C], mybir.dt.int8, kind="ExternalInput")
    p64_in = nc.dram_tensor("p64_in", [P, H], BF16, kind="ExternalInput")
    owt_in = nc.dram_tensor("owt_in", [H + 1, 2], F32, kind="ExternalInput")

    t_piece = nc.dram_tensor("t_piece", [NPIECE // 2, TROW], BF16)
    t_loc = nc.dram_tensor("t_loc", [CC // 2, TROW], BF16)
    bar_in = nc.dram_tensor("bar_in", [1, 16], I16)
    bar_out = nc.dram_tensor("bar_out", [2, 16], I16)
    t_full = nc.dram_tensor("t_full", [NPAIR + 1, TROW], BF16, addr_space="Shared")
    scores = nc.dram_tensor("scores", [BS, 2], F32, kind="ExternalOutput")

    APAIRS = ALPHA // 2           # 8192 pair-rows per alpha region
    CPAIRS = CC // 2              # 4608 pair-rows per cc region

    with tile.TileContext(nc) as tc:
        with tc.tile_pool(name="const", bufs=1) as cpool:
            w2_sb = cpool.tile([CROW, CHK * F], BF16, tag="w2")
            nc.sync.dma_start(
                w2_sb[:].rearrange("p (c f) -> p c f", c=CHK),
                bass.AP(w2, 0, [[F, CROW], [CROW * F, CHK], [1, F]]),
            )
            patch_sb = cpool.tile([1, TROW], F32, tag="patch")
            nc.sync.dma_start(patch_sb[:], patch[:, :])
            p64_sb = cpool.tile([P, H], BF16, tag="p64")
            owt_sb = cpool.tile([H + 1, 2], F32, tag="owt")
            idx_sb = cpool.tile([P, C // 16], I16, tag="idx")
            mv_sb = cpool.tile([P, C], mybir.dt.int8, tag="mv")

            preg = nc.sync.alloc_register("preg")
            nc.sync.reg_load(preg, par[0:1, 0:1])
            pv = nc.sync.snap(preg, donate=True, min_val=0, max_val=1)
            preg2 = nc.scalar.alloc_register("preg2")
            nc.scalar.reg_load(preg2, par[0:1, 0:1])
            pv2 = nc.scalar.snap(preg2, donate=True, min_val=0, max_val=1)
            preg3 = nc.gpsimd.alloc_register("preg3")
            nc.gpsimd.reg_load(preg3, par[0:1, 0:1])
            pv3 = nc.gpsimd.snap(preg3, donate=True, min_val=0, max_val=1)

            piece_sb = cpool.tile([P, PTILES * TROW], BF16, tag="piece")
            NG = ATILES // 8
            grp_sbs = []
            for g in range(NG):
                grp_t = cpool.tile([P, 8 * TROW], BF16, tag=f"grp{g}", name=f"grp{g}")
                grp_sbs.append(grp_t)

            # ---- Phase A: piece first (collective input), then alpha groups
            with (
                tc.tile_pool(name="pa", bufs=2) as papool,
                tc.tile_pool(name="pa_ps", bufs=3, space="PSUM") as paps,
            ):
                w2v = w2_sb[:].rearrange("p (c f) -> p c f", c=CHK)

                def build_slab(src_dram, ncols, t0, nt, out_sb, out_t0):
                    """Load an nt-tile slab and emit matmuls + copies."""
                    eu_t = papool.tile([CROW, CHK * nt * TROW], BF16, tag="eu_t")
                    euv = eu_t[:].rearrange("p (c j) -> p c j", c=CHK)
                    nc.sync.dma_start(
                        euv[:, :, 0 : nt * TROW],
                        bass.AP(
                            src_dram,
                            TROW * t0,
                            [[ncols, CROW], [CROW * ncols, CHK], [1, nt * TROW]],
                        ),
                    )
                    for i in range(nt):
                        acc = paps.tile([P, TROW], F32, tag="acc")
                        for c in range(CHK):
                            nc.tensor.matmul(
                                acc[:, 0:F],
                                lhsT=euv[:, c, i * TROW : i * TROW + P],
                                rhs=w2v[:, c, :],
                                start=(c == 0),
                                stop=(c == CHK - 1),
                            )
                        for c in range(CHK):
                            nc.tensor.matmul(
                                acc[:, F:TROW],
                                lhsT=euv[:, c, i * TROW + P : (i + 1) * TROW],
                                rhs=w2v[:, c, :],
                                start=(c == 0),
                                stop=(c == CHK - 1),
                            )
                        t = out_t0 + i
                        nc.vector.tensor_copy(
                            out_sb[:, t * TROW : (t + 1) * TROW], acc[:]
                        )

                t0 = 0
                while t0 < PTILES:
                    nt = min(2, PTILES - t0)
                    build_slab(eu_piece, NPIECE, t0, nt, piece_sb, t0)
                    t0 += nt
                nc.sync.dma_start(
                    bass.AP(t_piece, 0, [[TROW, P], [P * TROW, PTILES], [1, TROW]]),
                    piece_sb[:].rearrange("p (t j) -> p t j", t=PTILES),
                )
                nc.gpsimd.collective_compute(
                    "AllGather",
                    mybir.AluOpType.bypass,
                    replica_groups=[[0, 2, 4, 6], [1, 3, 5, 7]],
                    ins=[t_piece[:, :]],
                    outs=[t_loc[:, :]],
                )
                nc.vector.memset(idx_sb[:], 0)

                for g in range(NG):
                    if g == NG - 1:
                        build_slab(eu_alpha, ALPHA, 8 * g, 4, grp_sbs[g], 0)
                        build_slab(eu_alpha, ALPHA, 8 * g + 4, 4, grp_sbs[g], 4)
                    else:
                        build_slab(eu_alpha, ALPHA, 8 * g, 8, grp_sbs[g], 0)
                    if g == 2:
                        nc.sync.dma_start(p64_sb[:], p64_in[:, :])
                        nc.sync.dma_start(owt_sb[:], owt_in[:, :])
                    if g == 0:
                        nc.vector.tensor_copy(grp_sbs[0][0:1, 0:TROW], patch_sb[:])
                    src = grp_sbs[g][:].rearrange("p (t j) -> p t j", t=8)
                    for parity, base in ((0, 0), (1, 12800)):
                        nc.scalar.dma_start(
                            bass.AP(
                                t_full,
                                (base + g * 1024) * TROW,
                                [[TROW, P], [P * TROW, 8], [1, TROW]],
                            ),
                            src,
                            cond=(pv2 < 1) if parity == 0 else (pv2 > 0),
                        )

            pad0_sb = cpool.tile([1, TROW], BF16, tag="pad0")
            nc.vector.memset(pad0_sb[:], 0)
            nc.sync.dma_start(t_full[NPAIR : NPAIR + 1, :], pad0_sb[:])
            nc.sync.dma_start(idx_sb[0:32, :], idx_in[:, :])
            nc.sync.dma_start(mv_sb[:], mv_in[:, :])
            # collective part: bounce t_loc through SBUF into t_full cc region
            # (two pipelined halves)
            NCCG = CPAIRS // P
            NH = NCCG // 2
            cc_sb = cpool.tile([P, NCCG * TROW], BF16, tag="ccsb")
            ccv = cc_sb[:].rearrange("p (t j) -> p t j", t=NCCG)
            for h in range(2):
                nc.gpsimd.dma_start(
                    ccv[:, h * NH : (h + 1) * NH, :],
                    bass.AP(
                        t_loc,
                        h * NH * P * TROW,
                        [[TROW, P], [P * TROW, NH], [1, TROW]],
                    ),
                )
                for parity, base in ((0, APAIRS), (1, 12800 + APAIRS)):
                    nc.gpsimd.dma_start(
                        bass.AP(
                            t_full,
                            (base + h * NH * P) * TROW,
                            [[TROW, P], [P * TROW, NH], [1, TROW]],
                        ),
                        ccv[:, h * NH : (h + 1) * NH, :],
                        cond=(pv3 < 1) if parity == 0 else (pv3 > 0),
                    )

            # ---- barrier: probe one row of every written region, then 2-core
            # AllGather; partner's writes land before our gather.
            NPR = 13
            probe_sb = cpool.tile([2, NPR * 16], BF16, tag="probe")
            nc.sync.dma_start(
                probe_sb[:].rearrange("p (t j) -> p t j", t=NPR),
                bass.AP(
                    t_full, 0, [[12800 * TROW, 2], [1024 * TROW, NPR], [1, 16]]
                ),
            )
            nc.sync.dma_start(
                bar_in[:, :], probe_sb[:].bitcast(I16)[0:1, 0:16]
            )
            nc.gpsimd.collective_compute(
                "AllGather",
                mybir.AluOpType.bypass,
                replica_groups=[[0, 1], [2, 3], [4, 5], [6, 7]],
                ins=[bar_in[:, :]],
                outs=[bar_out[:, :]],
            )

            # ---- Phase B
            with (
                tc.tile_pool(name="pb", bufs=1) as pbpool,
                tc.tile_pool(name="pb_ps", bufs=4, space="PSUM") as pbps,
                tc.tile_pool(name="hd_ps", bufs=1, space="PSUM") as hdps,
            ):
                # dep: barrier -> gathers, via a write to t_full's pad row
                # (gather in_ap covers it, so both gathers acquire the RAW dep)
                nc.sync.dma_start(
                    t_full[NPAIR : NPAIR + 1, 0:16],
                    bar_out[0:1, :].bitcast(BF16),
                )

                NGC = 4
                Cg = C // NGC  # gather chunk (mult of 512)
                g_cs = []
                for ci in range(NGC):
                    g_c = pbpool.tile([P, 2 * Cg], BF16, tag=f"g{ci}", name=f"g{ci}")
                    g_cs.append(g_c)
                    nc.gpsimd.dma_gather(
                        out_ap=g_c[:].rearrange("p (two c) -> p two c", two=2),
                        in_ap=t_full[:, :],
                        idxs_ap=idx_sb[:, ci * Cg // 16 : (ci + 1) * Cg // 16],
                        num_idxs=Cg,
                        num_idxs_reg=Cg,
                        elem_size=TROW,
                        elem_step=TROW,
                        transpose=True,
                        queue_num=0,
                        single_packet=False,
                    )
                # in-place parity select: row pair [2r|2r+1] -> keep parity m
                gv = [
                    g_cs[ci][:].rearrange("p (two c) -> p two c", two=2)
                    for ci in range(NGC)
                ]
                for ci in range(NGC):
                    cs = slice(ci * Cg, (ci + 1) * Cg)
                    nc.vector.copy_predicated(
                        gv[ci][:, 0, :], mv_sb[:, cs], gv[ci][:, 1, :]
                    )

                def selcol(a, n=1):
                    ci = a // Cg
                    lo = a % Cg
                    return gv[ci][:, 0, lo : lo + n]

                # tap-1 partition extract via PE -> PSUM -> SBUF (Act), then
                # conv = selA + bsh[.+1] on Pool; reduces interleaved on DVE.
                conv = pbpool.tile([H, C], BF16, tag="conv")
                pooled = pbpool.tile([H + 1, BS], F32, tag="pooled")
                nc.vector.memset(pooled[H : H + 1, :], 1.0)
                zb = pbpool.tile([H, 1], F32, tag="zb")
                nc.vector.memset(zb[:], 0.0)
                bsh_sb = pbpool.tile([H, C], BF16, tag="bshsb")
                emitted = 0
                for q in range(NQ):
                    bsh = pbps.tile([H, 512], F32, tag="bsh")
                    nc.tensor.matmul(
                        bsh[:, :],
                        lhsT=p64_sb[:, :],
                        rhs=selcol(q * 512, 512),
                        start=True,
                        stop=True,
                    )
                    a0 = q * 512
                    nc.scalar.activation(
                        bsh_sb[:, a0 : a0 + 512],
                        bsh[:, :],
                        mybir.ActivationFunctionType.Identity,
                        bias=zb[:],
                    )
                    nc.gpsimd.tensor_tensor(
                        conv[:, a0 : a0 + 511],
                        selcol(a0, 511)[0:H],
                        bsh_sb[:, a0 + 1 : a0 + 512],
                        op=mybir.AluOpType.add,
                    )
                    if q > 0:
                        nc.gpsimd.tensor_tensor(
                            conv[:, a0 - 1 : a0],
                            selcol(a0 - 1)[0:H],
                            bsh_sb[:, a0 : a0 + 1],
                            op=mybir.AluOpType.add,
                        )
                    while emitted < BS and ranges[emitted][1] <= a0 + 511:
                        st, en = ranges[emitted]
                        nc.vector.reduce_max(
                            pooled[0:H, emitted : emitted + 1],
                            conv[:, st:en],
                            axis=mybir.AxisListType.X,
                        )
                        emitted += 1
                nc.vector.memset(conv[:, C - 1 : C], NEG)
                while emitted < BS:
                    st, en = ranges[emitted]
                    nc.vector.reduce_max(
                        pooled[0:H, emitted : emitted + 1],
                        conv[:, st:en],
                        axis=mybir.AxisListType.X,
                    )
                    emitted += 1

                sc_ps = hdps.tile([BS, 2], F32, tag="sc")
                nc.tensor.matmul(
                    sc_ps[:, :],
                    lhsT=pooled[:, :],
                    rhs=owt_sb[:, :],
                    start=True,
                    stop=True,
                )
                sc_sb = pbpool.tile([BS, 2], F32, tag="sc_sb")
                nc.vector.tensor_copy(sc_sb[:], sc_ps[:])
                nc.sync.dma_start(scores[:, :], sc_sb[:])

    nc.finalize()
    return nc


def prepare(sentences, E, U, conv_w, conv_b, out_w, out_b):
    """Host-side: shard/transpose/pack everything; returns (nc, in_maps, meta)."""
    sentences = np.asarray(sentences, dtype=np.int32)
    E = np.asarray(E, dtype=np.float32)
    U = np.asarray(U, dtype=np.float32)
    conv_w = np.asarray(conv_w, dtype=np.float32)
    conv_b = np.asarray(conv_b, dtype=np.float32)
    out_w = np.asarray(out_w, dtype=np.float32)
    out_b = np.asarray(out_b, dtype=np.float32)
    import ml_dtypes

    bf16 = ml_dtypes.bfloat16

    # ---- fused weight [600, 128]: w2[kd, 64k + h] = conv_w[h, kd, k]
    # [kd][k][h] -> col = k*H + h
    w2 = np.ascontiguousarray(conv_w.transpose(1, 2, 0).reshape(KD, 2 * H))

    # EU transposed, padded to VPAD rows: [600, VPAD] (row kd<300: E dim, else U)
    EU_T = np.zeros((KD, VPAD), dtype=bf16)
    EU_T[0:D, 0:V] = E.T.astype(bf16)
    EU_T[D:KD, 0:V] = U.T.astype(bf16)

    # halves: half-A rows [0, HALF), half-B rows [HALF, VPAD)
    # per-core slabs, per-tile even/odd interleaved columns
    def pack_cols(rows0, nrows):
        """cols for tiles covering vocab rows [rows0, rows0+nrows), per-tile:
        128 even rows then 128 odd rows."""
        ntile = nrows // 256
        cols = np.empty(nrows, dtype=np.int64)
        for t in range(ntile):
            base = rows0 + 256 * t
            cols[256 * t : 256 * t + 128] = base + 2 * np.arange(128)
            cols[256 * t + 128 : 256 * t + 256] = base + 2 * np.arange(128) + 1
        return cols

    # ---- ragged position lists, snake-balanced batch sharding
    lengths = np.sum(sentences != 1, axis=0)  # [B]
    T_b = np.minimum(lengths, S - 1)          # valid conv positions count
    n_ent_all = T_b + 1 + (lengths == S)      # + boundary + terminator
    order = np.argsort(-n_ent_all, kind="stable")  # rank-sorted sentence ids
    # slot b of core c <- order[8*b + c]
    assign = order.reshape(BS, NCORES)        # [slot, core]
    ne_slot = n_ent_all[assign].max(axis=1)   # slot-uniform entry counts
    csum = np.concatenate([[0], np.cumsum(ne_slot)])
    total = int(csum[-1])
    C = ((total + 16) + 2047) // 2048 * 2048  # mult of 2048, >=16 pad
    ranges = [(int(csum[b]), int(csum[b] + ne_slot[b])) for b in range(BS)]

    nc = build_nc(C, ranges)

    # host-computed patch rows
    def t_row(v):
        eu = np.concatenate([E[v], U[v]]).astype(np.float32)
        return eu @ w2  # [128]

    t1 = t_row(1)
    patch_even = np.empty((1, TROW), np.float32)
    patch_even[0, 0:F] = NEG                      # row 0: all -1e30
    patch_even[0, F : F + H] = NEG                # row 1 A-half
    patch_even[0, F + H : TROW] = t1[H:F]         # row 1 B-half = real
    patch_odd = np.empty((1, TROW), np.float32)
    patch_odd[0, 0:F] = t_row(HALF)
    patch_odd[0, F:TROW] = t_row(HALF + 1)

    p64 = np.zeros((P, H), dtype=bf16)
    p64[H:P, 0:H] = np.eye(H, dtype=bf16)

    owt = np.empty((H + 1, 2), np.float32)
    owt[0:H, :] = out_w.T
    owt[H, :] = out_b + out_w @ conv_b

    in_maps = []
    for c in range(NCORES):
        parity = c % 2
        k = c // 2
        half0 = 0 if parity == 0 else HALF
        # alpha slab: rows [half0, half0+ALPHA)
        a_cols = pack_cols(half0, ALPHA)
        eu_alpha = np.ascontiguousarray(EU_T[:, a_cols])
        # piece slab: rows [half0+ALPHA + k*NPIECE, +NPIECE)
        p_base = half0 + ALPHA + k * NPIECE
        p_cols = pack_cols(p_base, NPIECE)
        eu_piece = np.ascontiguousarray(EU_T[:, p_cols])

        # gather idx + parity masks
        sids = assign[:, c]                       # 32 sentence ids
        idx = np.zeros(C, np.int16)
        mvals = np.zeros(C, np.int8)
        for b in range(BS):
            sid = int(sids[b])
            st = int(csum[b])
            tb = int(T_b[sid])
            toks = sentences[0 : tb + 1, sid].astype(np.int64)  # positions 0..tb
            idx[st : st + tb + 1] = (toks >> 1).astype(np.int16)
            mvals[st : st + tb + 1] = (toks & 1).astype(np.int8)
            # rest of the slot (terminator and/or padding) stays idx 0, m 0
        wrapped = np.zeros((32, C // 16), np.int16)
        wr = idx.reshape(C // 16, 16).T           # [16, C/16]
        wrapped[0:16, :] = wr
        wrapped[16:32, :] = wr
        mv_full = np.broadcast_to(
            mvals.astype(np.int8)[None, :], (P, C)
        )
        in_maps.append(
            {
                "eu_alpha": eu_alpha,
                "eu_piece": eu_piece,
                "w2": w2.astype(bf16),
                "patch": patch_even if parity == 0 else patch_odd,
                "par": np.array([[parity]], np.int32),
                "idx_in": np.ascontiguousarray(wrapped),
                "mv_in": np.ascontiguousarray(mv_full),
                "p64_in": p64,
                "owt_in": owt,
            }
        )
    meta = {"assign": assign, "C": C}
    return nc, in_maps, meta


_CACHE = {}


def kernel(sentences, E, U, conv_w, conv_b, out_w, out_b):
    import hashlib

    h = hashlib.sha1()
    for a in (sentences, E, U, conv_w, conv_b, out_w, out_b):
        h.update(np.ascontiguousarray(a).tobytes())
    key = h.digest()
    if _CACHE.get("key") != key:
        nc, in_maps, meta = prepare(
            sentences, E, U, conv_w, conv_b, out_w, out_b
        )
        _CACHE.update(nc=nc, in_maps=in_maps, meta=meta, key=key)
    nc, in_maps, meta = _CACHE["nc"], _CACHE["in_maps"], _CACHE["meta"]
    res = run_bass_kernel_spmd(nc, in_maps, list(range(NCORES)))
    out = np.empty((B, 2), np.float32)
    assign = meta["assign"]
    for c in range(NCORES):
        sc = res.results[c]["scores"]
        out[assign[:, c]] = sc
    return out


# revision 30
# speedup vs baseline: 1.0197x; 1.0197x over previous
"""Trainium2 Bass kernel for the ragged text-CNN problem (v2).

Math: conv[b,h,t] = w0_h . e_{t,b} + w1_h . e_{t+1,b} + cb_h over valid t,
scores = (masked max_t conv) @ out_w.T + out_b, e = concat(E[tok], U[tok]).

Fused table T[v, 0:64] = e_v . w0, T[v, 64:128] = e_v . w1 (bf16), so
conv[b,h,t] = T[tok_t, h] + T[tok_{t+1}, 64+h].  PAD rows of T carry -1e30
on the tap-0 half, making the ragged mask free.

Distribution (8 cores, pair-shared HBM on (2k, 2k+1)):
- Table rows padded to V'=51200, stored pair-interleaved in a pair-shared
  DRAM tensor t_full [25600 pairs, 256] bf16.  Half-A (rows [0,25600)) is
  written by the even member, half-B by the odd member.
- Each member builds ALPHA=16384 rows of its half locally, plus a
  PIECE=2304-row shard of the remaining 9216 rows; two concurrent 4-core
  AllGathers (evens / odds) exchange the shards, then a DRAM->DRAM copy
  lands them in t_full.  A 2-core barrier collective orders the partner's
  writes before the gather.
- Phase B: one transposed dma_gather per position (512B pair fetch,
  idx = tok>>1 int16), parity select via copy_predicated, tap-1 shift via
  a PE partition-extract matmul, per-sentence reduce_max with
  slot-uniform compile-time ranges (host balances sentences by length).
"""

import numpy as np

try:
    import concourse.bass as bass
except ImportError:  # harness runs from a bare directory
    import sys

    sys.path.insert(0, "/opt/trn_rl_repo")
    import concourse.bass as bass

import concourse.mybir as mybir
from concourse.bacc import Bacc
import concourse.tile as tile
from concourse.bass_utils import run_bass_kernel_spmd

V, D, H, S, B = 50000, 300, 64, 512, 256
NCORES = 8
BS = B // NCORES            # sentences per core (32)
F = 2 * H                   # fused feature width (128)
KD = 2 * D                  # contraction size (600)
NEG = -1.0e30
P = 128

VPAD = 51200                # padded vocab (rows)
HALF = VPAD // 2            # rows per half (25600)
ALPHA = 20480               # locally-built rows per half
CC = HALF - ALPHA           # collective-delivered rows per half (5120)
NPIECE = CC // 4            # rows per core's collective shard (1280)
NPAIR = VPAD // 2           # pair-rows in t_full (25600)
TROW = 256                  # elems per pair-row (bf16) = 512B

ATILES = ALPHA // 256       # 64 tiles of 256 rows
PTILES = NPIECE // 256      # 9 tiles
CHK = 5                     # contraction chunks of 120 rows (5*120=600)
CROW = 120

F32 = mybir.dt.float32
BF16 = mybir.dt.bfloat16
I16 = mybir.dt.int16
I32 = mybir.dt.int32


def build_nc(C, ranges):
    """Per-core SPMD program.  C = gather positions (mult of 1024);
    ranges = 32 compile-time (start, end) column ranges, slot-uniform."""
    Cc = C // 2             # gather chunk (mult of 512)
    NQ = C // 512           # 512-col pipeline steps

    nc = Bacc()
    eu_alpha = nc.dram_tensor("eu_alpha", [KD, ALPHA], BF16, kind="ExternalInput")
    eu_piece = nc.dram_tensor("eu_piece", [KD, NPIECE], BF16, kind="ExternalInput")
    w2 = nc.dram_tensor("w2", [KD, F], BF16, kind="ExternalInput")
    patch = nc.dram_tensor("patch", [1, TROW], F32, kind="ExternalInput")
    par = nc.dram_tensor("par", [1, 1], I32, kind="ExternalInput")
    idx_in = nc.dram_tensor("idx_in", [32, C // 16], I16, kind="ExternalInput")
    mv_in = nc.dram_tensor("mv_in", [P, C], mybir.dt.int8, kind="ExternalInput")
    p64_in = nc.dram_tensor("p64_in", [P, H], BF16, kind="ExternalInput")
    owt_in = nc.dram_tensor("owt_in", [H + 1, 2], F32, kind="ExternalInput")

    t_piece = nc.dram_tensor("t_piece", [NPIECE // 2, TROW], BF16)
    t_loc = nc.dram_tensor("t_loc", [CC // 2, TROW], BF16)
    bar_in = nc.dram_tensor("bar_in", [1, 16], I16)
    bar_out = nc.dram_tensor("bar_out", [2, 16], I16)
    t_full = nc.dram_tensor("t_full", [NPAIR + 1, TROW], BF16, addr_space="Shared")
    scores = nc.dram_tensor("scores", [BS, 2], F32, kind="ExternalOutput")

    APAIRS = ALPHA // 2           # 8192 pair-rows per alpha region
    CPAIRS = CC // 2              # 4608 pair-rows per cc region

    with tile.TileContext(nc) as tc:
        with tc.tile_pool(name="const", bufs=1) as cpool:
            w2_sb = cpool.tile([CROW, CHK * F], BF16, tag="w2")
            nc.sync.dma_start(
                w2_sb[:].rearrange("p (c f) -> p c f", c=CHK),
                bass.AP(w2, 0, [[F, CROW], [CROW * F, CHK], [1, F]]),
            )
            patch_sb = cpool.tile([1, TROW], F32, tag="patch")
            nc.sync.dma_start(patch_sb[:], patch[:, :])
            p64_sb = cpool.tile([P, H], BF16, tag="p64")
            owt_sb = cpool.tile([H + 1, 2], F32, tag="owt")
            idx_sb = cpool.tile([P, C // 16], I16, tag="idx")
            mv_sb = cpool.tile([P, C], mybir.dt.int8, tag="mv")

            preg = nc.sync.alloc_register("preg")
            nc.sync.reg_load(preg, par[0:1, 0:1])
            pv = nc.sync.snap(preg, donate=True, min_val=0, max_val=1)
            preg2 = nc.scalar.alloc_register("preg2")
            nc.scalar.reg_load(preg2, par[0:1, 0:1])
            pv2 = nc.scalar.snap(preg2, donate=True, min_val=0, max_val=1)
            preg3 = nc.gpsimd.alloc_register("preg3")
            nc.gpsimd.reg_load(preg3, par[0:1, 0:1])
            pv3 = nc.gpsimd.snap(preg3, donate=True, min_val=0, max_val=1)

            piece_sb = cpool.tile([P, PTILES * TROW], BF16, tag="piece")
            NG = ATILES // 8
            grp_sbs = []
            for g in range(NG):
                grp_t = cpool.tile([P, 8 * TROW], BF16, tag=f"grp{g}", name=f"grp{g}")
                grp_sbs.append(grp_t)

            # ---- Phase A: piece first (collective input), then alpha groups
            with (
                tc.tile_pool(name="pa", bufs=2) as papool,
                tc.tile_pool(name="pa_ps", bufs=3, space="PSUM") as paps,
            ):
                w2v = w2_sb[:].rearrange("p (c f) -> p c f", c=CHK)

                def build_slab(src_dram, ncols, t0, nt, out_sb, out_t0):
                    """Load an nt-tile slab and emit matmuls + copies."""
                    eu_t = papool.tile([CROW, CHK * nt * TROW], BF16, tag="eu_t")
                    euv = eu_t[:].rearrange("p (c j) -> p c j", c=CHK)
                    nc.sync.dma_start(
                        euv[:, :, 0 : nt * TROW],
                        bass.AP(
                            src_dram,
                            TROW * t0,
                            [[ncols, CROW], [CROW * ncols, CHK], [1, nt * TROW]],
                        ),
                    )
                    for i in range(nt):
                        acc = paps.tile([P, TROW], F32, tag="acc")
                        for c in range(CHK):
                            nc.tensor.matmul(
                                acc[:, 0:F],
                                lhsT=euv[:, c, i * TROW : i * TROW + P],
                                rhs=w2v[:, c, :],
                                start=(c == 0),
                                stop=(c == CHK - 1),
                            )
                        for c in range(CHK):
                            nc.tensor.matmul(
                                acc[:, F:TROW],
                                lhsT=euv[:, c, i * TROW + P : (i + 1) * TROW],
                                rhs=w2v[:, c, :],
                                start=(c == 0),
                                stop=(c == CHK - 1),
                            )
                        t = out_t0 + i
                        nc.vector.tensor_copy(
                            out_sb[:, t * TROW : (t + 1) * TROW], acc[:]
                        )

                t0 = 0
                while t0 < PTILES:
                    nt = min(2, PTILES - t0)
                    build_slab(eu_piece, NPIECE, t0, nt, piece_sb, t0)
                    t0 += nt
                nc.sync.dma_start(
                    bass.AP(t_piece, 0, [[TROW, P], [P * TROW, PTILES], [1, TROW]]),
                    piece_sb[:].rearrange("p (t j) -> p t j", t=PTILES),
                )
                nc.gpsimd.collective_compute(
                    "AllGather",
                    mybir.AluOpType.bypass,
                    replica_groups=[[0, 2, 4, 6], [1, 3, 5, 7]],
                    ins=[t_piece[:, :]],
                    outs=[t_loc[:, :]],
                )
                nc.vector.memset(idx_sb[:], 0)

                for g in range(NG):
                    build_slab(eu_alpha, ALPHA, 8 * g, 8, grp_sbs[g], 0)
                    if g == 2:
                        nc.sync.dma_start(p64_sb[:], p64_in[:, :])
                        nc.sync.dma_start(owt_sb[:], owt_in[:, :])
                    if g == 0:
                        nc.vector.tensor_copy(grp_sbs[0][0:1, 0:TROW], patch_sb[:])
                    src = grp_sbs[g][:].rearrange("p (t j) -> p t j", t=8)
                    for parity, base in ((0, 0), (1, 12800)):
                        nc.scalar.dma_start(
                            bass.AP(
                                t_full,
                                (base + g * 1024) * TROW,
                                [[TROW, P], [P * TROW, 8], [1, TROW]],
                            ),
                            src,
                            cond=(pv2 < 1) if parity == 0 else (pv2 > 0),
                        )

            pad0_sb = cpool.tile([1, TROW], BF16, tag="pad0")
            nc.vector.memset(pad0_sb[:], 0)
            nc.sync.dma_start(t_full[NPAIR : NPAIR + 1, :], pad0_sb[:])
            nc.sync.dma_start(idx_sb[0:32, :], idx_in[:, :])
            nc.sync.dma_start(mv_sb[:], mv_in[:, :])
            # collective part: bounce t_loc through SBUF into t_full cc region
            # (two pipelined halves)
            NCCG = CPAIRS // P
            NH = NCCG // 2
            cc_sb = cpool.tile([P, NCCG * TROW], BF16, tag="ccsb")
            ccv = cc_sb[:].rearrange("p (t j) -> p t j", t=NCCG)
            for h in range(2):
                nc.gpsimd.dma_start(
                    ccv[:, h * NH : (h + 1) * NH, :],
                    bass.AP(
                        t_loc,
                        h * NH * P * TROW,
                        [[TROW, P], [P * TROW, NH], [1, TROW]],
                    ),
                )
                for parity, base in ((0, APAIRS), (1, 12800 + APAIRS)):
                    nc.gpsimd.dma_start(
                        bass.AP(
                            t_full,
                            (base + h * NH * P) * TROW,
                            [[TROW, P], [P * TROW, NH], [1, TROW]],
                        ),
                        ccv[:, h * NH : (h + 1) * NH, :],
                        cond=(pv3 < 1) if parity == 0 else (pv3 > 0),
                    )

            # ---- barrier: probe one row of every written region, then 2-core
            # AllGather; partner's writes land before our gather.
            NPR = 13
            probe_sb = cpool.tile([2, NPR * 16], BF16, tag="probe")
            nc.sync.dma_start(
                probe_sb[:].rearrange("p (t j) -> p t j", t=NPR),
                bass.AP(
                    t_full, 0, [[12800 * TROW, 2], [1024 * TROW, NPR], [1, 16]]
                ),
            )
            nc.sync.dma_start(
                bar_in[:, :], probe_sb[:].bitcast(I16)[0:1, 0:16]
            )
            nc.gpsimd.collective_compute(
                "AllGather",
                mybir.AluOpType.bypass,
                replica_groups=[[0, 1], [2, 3], [4, 5], [6, 7]],
                ins=[bar_in[:, :]],
                outs=[bar_out[:, :]],
            )

            # ---- Phase B
            with (
                tc.tile_pool(name="pb", bufs=1) as pbpool,
                tc.tile_pool(name="pb_ps", bufs=4, space="PSUM") as pbps,
                tc.tile_pool(name="hd_ps", bufs=1, space="PSUM") as hdps,
            ):
                # dep: barrier -> gathers, via a write to t_full's pad row
                # (gather in_ap covers it, so both gathers acquire the RAW dep)
                nc.sync.dma_start(
                    t_full[NPAIR : NPAIR + 1, 0:16],
                    bar_out[0:1, :].bitcast(BF16),
                )

                NGC = 4
                Cg = C // NGC  # gather chunk (mult of 512)
                g_cs = []
                for ci in range(NGC):
                    g_c = pbpool.tile([P, 2 * Cg], BF16, tag=f"g{ci}", name=f"g{ci}")
                    g_cs.append(g_c)
                    nc.gpsimd.dma_gather(
                        out_ap=g_c[:].rearrange("p (two c) -> p two c", two=2),
                        in_ap=t_full[:, :],
                        idxs_ap=idx_sb[:, ci * Cg // 16 : (ci + 1) * Cg // 16],
                        num_idxs=Cg,
                        num_idxs_reg=Cg,
                        elem_size=TROW,
                        elem_step=TROW,
                        transpose=True,
                        queue_num=0,
                        single_packet=False,
                    )
                # in-place parity select: row pair [2r|2r+1] -> keep parity m
                gv = [
                    g_cs[ci][:].rearrange("p (two c) -> p two c", two=2)
                    for ci in range(NGC)
                ]
                for ci in range(NGC):
                    cs = slice(ci * Cg, (ci + 1) * Cg)
                    nc.vector.copy_predicated(
                        gv[ci][:, 0, :], mv_sb[:, cs], gv[ci][:, 1, :]
                    )

                def selcol(a, n=1):
                    ci = a // Cg
                    lo = a % Cg
                    return gv[ci][:, 0, lo : lo + n]

                # tap-1 partition extract via PE -> PSUM -> SBUF (Act), then
                # conv = selA + bsh[.+1] on Pool; reduces interleaved on DVE.
                conv = pbpool.tile([H, C], BF16, tag="conv")
                pooled = pbpool.tile([H + 1, BS], F32, tag="pooled")
                nc.vector.memset(pooled[H : H + 1, :], 1.0)
                zb = pbpool.tile([H, 1], F32, tag="zb")
                nc.vector.memset(zb[:], 0.0)
                bsh_sb = pbpool.tile([H, C], BF16, tag="bshsb")
                emitted = 0
                for q in range(NQ):
                    bsh = pbps.tile([H, 512], F32, tag="bsh")
                    nc.tensor.matmul(
                        bsh[:, :],
                        lhsT=p64_sb[:, :],
                        rhs=selcol(q * 512, 512),
                        start=True,
                        stop=True,
                    )
                    a0 = q * 512
                    nc.scalar.activation(
                        bsh_sb[:, a0 : a0 + 512],
                        bsh[:, :],
                        mybir.ActivationFunctionType.Identity,
                        bias=zb[:],
                    )
                    nc.gpsimd.tensor_tensor(
                        conv[:, a0 : a0 + 511],
                        selcol(a0, 511)[0:H],
                        bsh_sb[:, a0 + 1 : a0 + 512],
                        op=mybir.AluOpType.add,
                    )
                    if q > 0:
                        nc.gpsimd.tensor_tensor(
                            conv[:, a0 - 1 : a0],
                            selcol(a0 - 1)[0:H],
                            bsh_sb[:, a0 : a0 + 1],
                            op=mybir.AluOpType.add,
                        )
                    while emitted < BS and ranges[emitted][1] <= a0 + 511:
                        st, en = ranges[emitted]
                        nc.vector.reduce_max(
                            pooled[0:H, emitted : emitted + 1],
                            conv[:, st:en],
                            axis=mybir.AxisListType.X,
                        )
                        emitted += 1
                nc.vector.memset(conv[:, C - 1 : C], NEG)
                while emitted < BS:
                    st, en = ranges[emitted]
                    nc.vector.reduce_max(
                        pooled[0:H, emitted : emitted + 1],
                        conv[:, st:en],
                        axis=mybir.AxisListType.X,
                    )
                    emitted += 1

                sc_ps = hdps.tile([BS, 2], F32, tag="sc")
                nc.tensor.matmul(
                    sc_ps[:, :],
                    lhsT=pooled[:, :],
                    rhs=owt_sb[:, :],
                    start=True,
                    stop=True,
                )
                sc_sb = pbpool.tile([BS, 2], F32, tag="sc_sb")
                nc.vector.tensor_copy(sc_sb[:], sc_ps[:])
                nc.sync.dma_start(scores[:, :], sc_sb[:])

    nc.finalize()
    return nc


def prepare(sentences, E, U, conv_w, conv_b, out_w, out_b):
    """Host-side: shard/transpose/pack everything; returns (nc, in_maps, meta)."""
    sentences = np.asarray(sentences, dtype=np.int32)
    E = np.asarray(E, dtype=np.float32)
    U = np.asarray(U, dtype=np.float32)
    conv_w = np.asarray(conv_w, dtype=np.float32)
    conv_b = np.asarray(conv_b, dtype=np.float32)
    out_w = np.asarray(out_w, dtype=np.float32)
    out_b = np.asarray(out_b, dtype=np.float32)
    import ml_dtypes

    bf16 = ml_dtypes.bfloat16

    # ---- fused weight [600, 128]: w2[kd, 64k + h] = conv_w[h, kd, k]
    # [kd][k][h] -> col = k*H + h
    w2 = np.ascontiguousarray(conv_w.transpose(1, 2, 0).reshape(KD, 2 * H))

    # EU transposed, padded to VPAD rows: [600, VPAD] (row kd<300: E dim, else U)
    EU_T = np.zeros((KD, VPAD), dtype=bf16)
    EU_T[0:D, 0:V] = E.T.astype(bf16)
    EU_T[D:KD, 0:V] = U.T.astype(bf16)

    # halves: half-A rows [0, HALF), half-B rows [HALF, VPAD)
    # per-core slabs, per-tile even/odd interleaved columns
    def pack_cols(rows0, nrows):
        """cols for tiles covering vocab rows [rows0, rows0+nrows), per-tile:
        128 even rows then 128 odd rows."""
        ntile = nrows // 256
        cols = np.empty(nrows, dtype=np.int64)
        for t in range(ntile):
            base = rows0 + 256 * t
            cols[256 * t : 256 * t + 128] = base + 2 * np.arange(128)
            cols[256 * t + 128 : 256 * t + 256] = base + 2 * np.arange(128) + 1
        return cols

    # ---- ragged position lists, snake-balanced batch sharding
    lengths = np.sum(sentences != 1, axis=0)  # [B]
    T_b = np.minimum(lengths, S - 1)          # valid conv positions count
    n_ent_all = T_b + 1 + (lengths == S)      # + boundary + terminator
    order = np.argsort(-n_ent_all, kind="stable")  # rank-sorted sentence ids
    # slot b of core c <- order[8*b + c]
    assign = order.reshape(BS, NCORES)        # [slot, core]
    ne_slot = n_ent_all[assign].max(axis=1)   # slot-uniform entry counts
    csum = np.concatenate([[0], np.cumsum(ne_slot)])
    total = int(csum[-1])
    C = ((total + 16) + 2047) // 2048 * 2048  # mult of 2048, >=16 pad
    ranges = [(int(csum[b]), int(csum[b] + ne_slot[b])) for b in range(BS)]

    nc = build_nc(C, ranges)

    # host-computed patch rows
    def t_row(v):
        eu = np.concatenate([E[v], U[v]]).astype(np.float32)
        return eu @ w2  # [128]

    t1 = t_row(1)
    patch_even = np.empty((1, TROW), np.float32)
    patch_even[0, 0:F] = NEG                      # row 0: all -1e30
    patch_even[0, F : F + H] = NEG                # row 1 A-half
    patch_even[0, F + H : TROW] = t1[H:F]         # row 1 B-half = real
    patch_odd = np.empty((1, TROW), np.float32)
    patch_odd[0, 0:F] = t_row(HALF)
    patch_odd[0, F:TROW] = t_row(HALF + 1)

    p64 = np.zeros((P, H), dtype=bf16)
    p64[H:P, 0:H] = np.eye(H, dtype=bf16)

    owt = np.empty((H + 1, 2), np.float32)
    owt[0:H, :] = out_w.T
    owt[H, :] = out_b + out_w @ conv_b

    in_maps = []
    for c in range(NCORES):
        parity = c % 2
        k = c // 2
        half0 = 0 if parity == 0 else HALF
        # alpha slab: rows [half0, half0+ALPHA)
        a_cols = pack_cols(half0, ALPHA)
        eu_alpha = np.ascontiguousarray(EU_T[:, a_cols])
        # piece slab: rows [half0+ALPHA + k*NPIECE, +NPIECE)
        p_base = half0 + ALPHA + k * NPIECE
        p_cols = pack_cols(p_base, NPIECE)
        eu_piece = np.ascontiguousarray(EU_T[:, p_cols])

        # gather idx + parity masks
        sids = assign[:, c]                       # 32 sentence ids
        idx = np.zeros(C, np.int16)
        mvals = np.zeros(C, np.int8)
        for b in range(BS):
            sid = int(sids[b])
            st = int(csum[b])
            tb = int(T_b[sid])
            toks = sentences[0 : tb + 1, sid].astype(np.int64)  # positions 0..tb
            idx[st : st + tb + 1] = (toks >> 1).astype(np.int16)
            mvals[st : st + tb + 1] = (toks & 1).astype(np.int8)
            # rest of the slot (terminator and/or padding) stays idx 0, m 0
        wrapped = np.zeros((32, C // 16), np.int16)
        wr = idx.reshape(C // 16, 16).T           # [16, C/16]
        wrapped[0:16, :] = wr
        wrapped[16:32, :] = wr
        mv_full = np.broadcast_to(
            mvals.astype(np.int8)[None, :], (P, C)
        )
        in_maps.append(
            {
                "eu_alpha": eu_alpha,
                "eu_piece": eu_piece,
                "w2": w2.astype(bf16),
                "patch": patch_even if parity == 0 else patch_odd,
                "par": np.array([[parity]], np.int32),
                "idx_in": np.ascontiguousarray(wrapped),
                "mv_in": np.ascontiguousarray(mv_full),
                "p64_in": p64,
                "owt_in": owt,
            }
        )
    meta = {"assign": assign, "C": C}
    return nc, in_maps, meta


_CACHE = {}


def kernel(sentences, E, U, conv_w, conv_b, out_w, out_b):
    import hashlib

    h = hashlib.sha1()
    for a in (sentences, E, U, conv_w, conv_b, out_w, out_b):
        h.update(np.ascontiguousarray(a).tobytes())
    key = h.digest()
    if _CACHE.get("key") != key:
        nc, in_maps, meta = prepare(
            sentences, E, U, conv_w, conv_b, out_w, out_b
        )
        _CACHE.update(nc=nc, in_maps=in_maps, meta=meta, key=key)
    nc, in_maps, meta = _CACHE["nc"], _CACHE["in_maps"], _CACHE["meta"]
    res = run_bass_kernel_spmd(nc, in_maps, list(range(NCORES)))
    out = np.empty((B, 2), np.float32)
    assign = meta["assign"]
    for c in range(NCORES):
        sc = res.results[c]["scores"]
        out[assign[:, c]] = sc
    return out


# revision 31
# speedup vs baseline: 1.0298x; 1.0099x over previous
"""Trainium2 Bass kernel for the ragged text-CNN problem (v2).

Math: conv[b,h,t] = w0_h . e_{t,b} + w1_h . e_{t+1,b} + cb_h over valid t,
scores = (masked max_t conv) @ out_w.T + out_b, e = concat(E[tok], U[tok]).

Fused table T[v, 0:64] = e_v . w0, T[v, 64:128] = e_v . w1 (bf16), so
conv[b,h,t] = T[tok_t, h] + T[tok_{t+1}, 64+h].  PAD rows of T carry -1e30
on the tap-0 half, making the ragged mask free.

Distribution (8 cores, pair-shared HBM on (2k, 2k+1)):
- Table rows padded to V'=51200, stored pair-interleaved in a pair-shared
  DRAM tensor t_full [25600 pairs, 256] bf16.  Half-A (rows [0,25600)) is
  written by the even member, half-B by the odd member.
- Each member builds ALPHA=16384 rows of its half locally, plus a
  PIECE=2304-row shard of the remaining 9216 rows; two concurrent 4-core
  AllGathers (evens / odds) exchange the shards, then a DRAM->DRAM copy
  lands them in t_full.  A 2-core barrier collective orders the partner's
  writes before the gather.
- Phase B: one transposed dma_gather per position (512B pair fetch,
  idx = tok>>1 int16), parity select via copy_predicated, tap-1 shift via
  a PE partition-extract matmul, per-sentence reduce_max with
  slot-uniform compile-time ranges (host balances sentences by length).
"""

import numpy as np

try:
    import concourse.bass as bass
except ImportError:  # harness runs from a bare directory
    import sys

    sys.path.insert(0, "/opt/trn_rl_repo")
    import concourse.bass as bass

import concourse.mybir as mybir
from concourse.bacc import Bacc
import concourse.tile as tile
from concourse.bass_utils import run_bass_kernel_spmd

V, D, H, S, B = 50000, 300, 64, 512, 256
NCORES = 8
BS = B // NCORES            # sentences per core (32)
F = 2 * H                   # fused feature width (128)
KD = 2 * D                  # contraction size (600)
NEG = -1.0e30
P = 128

VPAD = 51200                # padded vocab (rows)
HALF = VPAD // 2            # rows per half (25600)
ALPHA = 20480               # locally-built rows per half
CC = HALF - ALPHA           # collective-delivered rows per half (5120)
NPIECE = CC // 4            # rows per core's collective shard (1280)
NPAIR = VPAD // 2           # pair-rows in t_full (25600)
TROW = 256                  # elems per pair-row (bf16) = 512B

ATILES = ALPHA // 256       # 64 tiles of 256 rows
PTILES = NPIECE // 256      # 9 tiles
CHK = 5                     # contraction chunks of 120 rows (5*120=600)
CROW = 120

F32 = mybir.dt.float32
BF16 = mybir.dt.bfloat16
I16 = mybir.dt.int16
I32 = mybir.dt.int32


def build_nc(C, ranges):
    """Per-core SPMD program.  C = gather positions (mult of 1024);
    ranges = 32 compile-time (start, end) column ranges, slot-uniform."""
    Cc = C // 2             # gather chunk (mult of 512)
    NQ = C // 512           # 512-col pipeline steps

    nc = Bacc()
    eu_alpha = nc.dram_tensor("eu_alpha", [KD, ALPHA], BF16, kind="ExternalInput")
    eu_piece = nc.dram_tensor("eu_piece", [KD, NPIECE], BF16, kind="ExternalInput")
    w2 = nc.dram_tensor("w2", [KD, F], BF16, kind="ExternalInput")
    patch = nc.dram_tensor("patch", [1, TROW], F32, kind="ExternalInput")
    par = nc.dram_tensor("par", [1, 1], I32, kind="ExternalInput")
    idx_in = nc.dram_tensor("idx_in", [32, C // 16], I16, kind="ExternalInput")
    mv_in = nc.dram_tensor("mv_in", [P, C], mybir.dt.int8, kind="ExternalInput")
    p64_in = nc.dram_tensor("p64_in", [P, H], BF16, kind="ExternalInput")
    owt_in = nc.dram_tensor("owt_in", [H + 1, 2], F32, kind="ExternalInput")

    t_piece = nc.dram_tensor("t_piece", [NPIECE // 2, TROW], BF16)
    t_loc = nc.dram_tensor("t_loc", [CC // 2, TROW], BF16)
    bar_in = nc.dram_tensor("bar_in", [1, 16], I16)
    bar_out = nc.dram_tensor("bar_out", [2, 16], I16)
    t_full = nc.dram_tensor("t_full", [NPAIR + 1, TROW], BF16, addr_space="Shared")
    scores = nc.dram_tensor("scores", [BS, 2], F32, kind="ExternalOutput")

    APAIRS = ALPHA // 2           # 8192 pair-rows per alpha region
    CPAIRS = CC // 2              # 4608 pair-rows per cc region

    with tile.TileContext(nc) as tc:
        with tc.tile_pool(name="const", bufs=1) as cpool:
            w2_sb = cpool.tile([CROW, CHK * F], BF16, tag="w2")
            nc.sync.dma_start(
                w2_sb[:].rearrange("p (c f) -> p c f", c=CHK),
                bass.AP(w2, 0, [[F, CROW], [CROW * F, CHK], [1, F]]),
            )
            patch_sb = cpool.tile([1, TROW], F32, tag="patch")
            nc.sync.dma_start(patch_sb[:], patch[:, :])
            p64_sb = cpool.tile([P, H], BF16, tag="p64")
            owt_sb = cpool.tile([H + 1, 2], F32, tag="owt")
            idx_sb = cpool.tile([P, C // 16], I16, tag="idx")
            mv_sb = cpool.tile([P, C], mybir.dt.int8, tag="mv")

            preg = nc.sync.alloc_register("preg")
            nc.sync.reg_load(preg, par[0:1, 0:1])
            pv = nc.sync.snap(preg, donate=True, min_val=0, max_val=1)
            preg2 = nc.scalar.alloc_register("preg2")
            nc.scalar.reg_load(preg2, par[0:1, 0:1])
            pv2 = nc.scalar.snap(preg2, donate=True, min_val=0, max_val=1)
            preg3 = nc.gpsimd.alloc_register("preg3")
            nc.gpsimd.reg_load(preg3, par[0:1, 0:1])
            pv3 = nc.gpsimd.snap(preg3, donate=True, min_val=0, max_val=1)

            piece_sb = cpool.tile([P, PTILES * TROW], BF16, tag="piece")
            NG = ATILES // 8
            grp_sbs = []
            for g in range(NG):
                grp_t = cpool.tile([P, 8 * TROW], BF16, tag=f"grp{g}", name=f"grp{g}")
                grp_sbs.append(grp_t)

            # ---- Phase A: piece first (collective input), then alpha groups
            with (
                tc.tile_pool(name="pa", bufs=2) as papool,
                tc.tile_pool(name="pa_ps", bufs=3, space="PSUM") as paps,
            ):
                w2v = w2_sb[:].rearrange("p (c f) -> p c f", c=CHK)

                def build_slab(src_dram, ncols, t0, nt, out_sb, out_t0):
                    """Load an nt-tile slab and emit matmuls + copies."""
                    eu_t = papool.tile([CROW, CHK * nt * TROW], BF16, tag="eu_t")
                    euv = eu_t[:].rearrange("p (c j) -> p c j", c=CHK)
                    nc.sync.dma_start(
                        euv[:, :, 0 : nt * TROW],
                        bass.AP(
                            src_dram,
                            TROW * t0,
                            [[ncols, CROW], [CROW * ncols, CHK], [1, nt * TROW]],
                        ),
                    )
                    for i in range(nt):
                        acc = paps.tile([P, TROW], F32, tag="acc")
                        for c in range(CHK):
                            nc.tensor.matmul(
                                acc[:, 0:F],
                                lhsT=euv[:, c, i * TROW : i * TROW + P],
                                rhs=w2v[:, c, :],
                                start=(c == 0),
                                stop=(c == CHK - 1),
                            )
                        for c in range(CHK):
                            nc.tensor.matmul(
                                acc[:, F:TROW],
                                lhsT=euv[:, c, i * TROW + P : (i + 1) * TROW],
                                rhs=w2v[:, c, :],
                                start=(c == 0),
                                stop=(c == CHK - 1),
                            )
                        t = out_t0 + i
                        nc.vector.tensor_copy(
                            out_sb[:, t * TROW : (t + 1) * TROW], acc[:]
                        )

                t0 = 0
                while t0 < PTILES:
                    nt = min(2, PTILES - t0)
                    build_slab(eu_piece, NPIECE, t0, nt, piece_sb, t0)
                    t0 += nt
                nc.sync.dma_start(
                    bass.AP(t_piece, 0, [[TROW, P], [P * TROW, PTILES], [1, TROW]]),
                    piece_sb[:].rearrange("p (t j) -> p t j", t=PTILES),
                )
                nc.gpsimd.collective_compute(
                    "AllGather",
                    mybir.AluOpType.bypass,
                    replica_groups=[[0, 2, 4, 6], [1, 3, 5, 7]],
                    ins=[t_piece[:, :]],
                    outs=[t_loc[:, :]],
                )
                nc.vector.memset(idx_sb[:], 0)

                for g in range(NG):
                    build_slab(eu_alpha, ALPHA, 8 * g, 8, grp_sbs[g], 0)
                    if g == 2:
                        nc.sync.dma_start(p64_sb[:], p64_in[:, :])
                        nc.sync.dma_start(owt_sb[:], owt_in[:, :])
                    if g == 0:
                        nc.vector.tensor_copy(grp_sbs[0][0:1, 0:TROW], patch_sb[:])
                    src = grp_sbs[g][:].rearrange("p (t j) -> p t j", t=8)
                    for parity, base in ((0, 0), (1, 12800)):
                        nc.scalar.dma_start(
                            bass.AP(
                                t_full,
                                (base + g * 1024) * TROW,
                                [[TROW, P], [P * TROW, 8], [1, TROW]],
                            ),
                            src,
                            cond=(pv2 < 1) if parity == 0 else (pv2 > 0),
                        )

            pad0_sb = cpool.tile([1, TROW], BF16, tag="pad0")
            nc.vector.memset(pad0_sb[:], 0)
            nc.sync.dma_start(t_full[NPAIR : NPAIR + 1, :], pad0_sb[:])
            nc.sync.dma_start(idx_sb[0:32, :], idx_in[:, :])
            nc.sync.dma_start(mv_sb[:], mv_in[:, :])
            # collective part: bounce t_loc through SBUF into t_full cc region
            # (two pipelined halves)
            NCCG = CPAIRS // P
            NH = NCCG // 2
            cc_sb = cpool.tile([P, NCCG * TROW], BF16, tag="ccsb")
            ccv = cc_sb[:].rearrange("p (t j) -> p t j", t=NCCG)
            for h in range(2):
                nc.gpsimd.dma_start(
                    ccv[:, h * NH : (h + 1) * NH, :],
                    bass.AP(
                        t_loc,
                        h * NH * P * TROW,
                        [[TROW, P], [P * TROW, NH], [1, TROW]],
                    ),
                )
                for parity, base in ((0, APAIRS), (1, 12800 + APAIRS)):
                    nc.gpsimd.dma_start(
                        bass.AP(
                            t_full,
                            (base + h * NH * P) * TROW,
                            [[TROW, P], [P * TROW, NH], [1, TROW]],
                        ),
                        ccv[:, h * NH : (h + 1) * NH, :],
                        cond=(pv3 < 1) if parity == 0 else (pv3 > 0),
                    )

            # ---- barrier: probe one row of every written region, then 2-core
            # AllGather; partner's writes land before our gather.
            NPR = 13
            probe_sb = cpool.tile([2, NPR * 16], BF16, tag="probe")
            nc.sync.dma_start(
                probe_sb[:].rearrange("p (t j) -> p t j", t=NPR),
                bass.AP(
                    t_full, 0, [[12800 * TROW, 2], [1024 * TROW, NPR], [1, 16]]
                ),
            )
            nc.sync.dma_start(
                bar_in[:, :], probe_sb[:].bitcast(I16)[0:1, 0:16]
            )
            nc.gpsimd.collective_compute(
                "AllGather",
                mybir.AluOpType.bypass,
                replica_groups=[[0, 1], [2, 3], [4, 5], [6, 7]],
                ins=[bar_in[:, :]],
                outs=[bar_out[:, :]],
            )

            # ---- Phase B
            with (
                tc.tile_pool(name="pb", bufs=1) as pbpool,
                tc.tile_pool(name="pb_ps", bufs=4, space="PSUM") as pbps,
                tc.tile_pool(name="hd_ps", bufs=1, space="PSUM") as hdps,
            ):
                # dep: barrier -> gathers, via a write to t_full's pad row
                # (gather in_ap covers it, so both gathers acquire the RAW dep)
                nc.sync.dma_start(
                    t_full[NPAIR : NPAIR + 1, 0:16],
                    bar_out[0:1, :].bitcast(BF16),
                )

                NGC = 3
                Cg = C // NGC  # gather chunk (mult of 512)
                g_cs = []
                for ci in range(NGC):
                    g_c = pbpool.tile([P, 2 * Cg], BF16, tag=f"g{ci}", name=f"g{ci}")
                    g_cs.append(g_c)
                    nc.gpsimd.dma_gather(
                        out_ap=g_c[:].rearrange("p (two c) -> p two c", two=2),
                        in_ap=t_full[:, :],
                        idxs_ap=idx_sb[:, ci * Cg // 16 : (ci + 1) * Cg // 16],
                        num_idxs=Cg,
                        num_idxs_reg=Cg,
                        elem_size=TROW,
                        elem_step=TROW,
                        transpose=True,
                        queue_num=0,
                        single_packet=False,
                    )
                # in-place parity select: row pair [2r|2r+1] -> keep parity m
                gv = [
                    g_cs[ci][:].rearrange("p (two c) -> p two c", two=2)
                    for ci in range(NGC)
                ]
                for ci in range(NGC):
                    cs = slice(ci * Cg, (ci + 1) * Cg)
                    nc.vector.copy_predicated(
                        gv[ci][:, 0, :], mv_sb[:, cs], gv[ci][:, 1, :]
                    )

                def selcol(a, n=1):
                    ci = a // Cg
                    lo = a % Cg
                    return gv[ci][:, 0, lo : lo + n]

                # tap-1 partition extract via PE -> PSUM -> SBUF (Act), then
                # conv = selA + bsh[.+1] on Pool; reduces interleaved on DVE.
                conv = pbpool.tile([H, C], BF16, tag="conv")
                pooled = pbpool.tile([H + 1, BS], F32, tag="pooled")
                nc.vector.memset(pooled[H : H + 1, :], 1.0)
                zb = pbpool.tile([H, 1], F32, tag="zb")
                nc.vector.memset(zb[:], 0.0)
                bsh_sb = pbpool.tile([H, C], BF16, tag="bshsb")
                emitted = 0
                for q in range(NQ):
                    bsh = pbps.tile([H, 512], F32, tag="bsh")
                    nc.tensor.matmul(
                        bsh[:, :],
                        lhsT=p64_sb[:, :],
                        rhs=selcol(q * 512, 512),
                        start=True,
                        stop=True,
                    )
                    a0 = q * 512
                    nc.scalar.activation(
                        bsh_sb[:, a0 : a0 + 512],
                        bsh[:, :],
                        mybir.ActivationFunctionType.Identity,
                        bias=zb[:],
                    )
                    nc.gpsimd.tensor_tensor(
                        conv[:, a0 : a0 + 511],
                        selcol(a0, 511)[0:H],
                        bsh_sb[:, a0 + 1 : a0 + 512],
                        op=mybir.AluOpType.add,
                    )
                    if q > 0:
                        nc.gpsimd.tensor_tensor(
                            conv[:, a0 - 1 : a0],
                            selcol(a0 - 1)[0:H],
                            bsh_sb[:, a0 : a0 + 1],
                            op=mybir.AluOpType.add,
                        )
                    while emitted < BS and ranges[emitted][1] <= a0 + 511:
                        st, en = ranges[emitted]
                        nc.vector.reduce_max(
                            pooled[0:H, emitted : emitted + 1],
                            conv[:, st:en],
                            axis=mybir.AxisListType.X,
                        )
                        emitted += 1
                nc.vector.memset(conv[:, C - 1 : C], NEG)
                while emitted < BS:
                    st, en = ranges[emitted]
                    nc.vector.reduce_max(
                        pooled[0:H, emitted : emitted + 1],
                        conv[:, st:en],
                        axis=mybir.AxisListType.X,
                    )
                    emitted += 1

                sc_ps = hdps.tile([BS, 2], F32, tag="sc")
                nc.tensor.matmul(
                    sc_ps[:, :],
                    lhsT=pooled[:, :],
                    rhs=owt_sb[:, :],
                    start=True,
                    stop=True,
                )
                sc_sb = pbpool.tile([BS, 2], F32, tag="sc_sb")
                nc.vector.tensor_copy(sc_sb[:], sc_ps[:])
                nc.sync.dma_start(scores[:, :], sc_sb[:])

    nc.finalize()
    return nc


def prepare(sentences, E, U, conv_w, conv_b, out_w, out_b):
    """Host-side: shard/transpose/pack everything; returns (nc, in_maps, meta)."""
    sentences = np.asarray(sentences, dtype=np.int32)
    E = np.asarray(E, dtype=np.float32)
    U = np.asarray(U, dtype=np.float32)
    conv_w = np.asarray(conv_w, dtype=np.float32)
    conv_b = np.asarray(conv_b, dtype=np.float32)
    out_w = np.asarray(out_w, dtype=np.float32)
    out_b = np.asarray(out_b, dtype=np.float32)
    import ml_dtypes

    bf16 = ml_dtypes.bfloat16

    # ---- fused weight [600, 128]: w2[kd, 64k + h] = conv_w[h, kd, k]
    # [kd][k][h] -> col = k*H + h
    w2 = np.ascontiguousarray(conv_w.transpose(1, 2, 0).reshape(KD, 2 * H))

    # EU transposed, padded to VPAD rows: [600, VPAD] (row kd<300: E dim, else U)
    EU_T = np.zeros((KD, VPAD), dtype=bf16)
    EU_T[0:D, 0:V] = E.T.astype(bf16)
    EU_T[D:KD, 0:V] = U.T.astype(bf16)

    # halves: half-A rows [0, HALF), half-B rows [HALF, VPAD)
    # per-core slabs, per-tile even/odd interleaved columns
    def pack_cols(rows0, nrows):
        """cols for tiles covering vocab rows [rows0, rows0+nrows), per-tile:
        128 even rows then 128 odd rows."""
        ntile = nrows // 256
        cols = np.empty(nrows, dtype=np.int64)
        for t in range(ntile):
            base = rows0 + 256 * t
            cols[256 * t : 256 * t + 128] = base + 2 * np.arange(128)
            cols[256 * t + 128 : 256 * t + 256] = base + 2 * np.arange(128) + 1
        return cols

    # ---- ragged position lists, snake-balanced batch sharding
    lengths = np.sum(sentences != 1, axis=0)  # [B]
    T_b = np.minimum(lengths, S - 1)          # valid conv positions count
    n_ent_all = T_b + 1 + (lengths == S)      # + boundary + terminator
    order = np.argsort(-n_ent_all, kind="stable")  # rank-sorted sentence ids
    # slot b of core c <- order[8*b + c]
    assign = order.reshape(BS, NCORES)        # [slot, core]
    ne_slot = n_ent_all[assign].max(axis=1)   # slot-uniform entry counts
    csum = np.concatenate([[0], np.cumsum(ne_slot)])
    total = int(csum[-1])
    C = ((total + 16) + 1535) // 1536 * 1536  # mult of 3*512, >=16 pad
    ranges = [(int(csum[b]), int(csum[b] + ne_slot[b])) for b in range(BS)]

    nc = build_nc(C, ranges)

    # host-computed patch rows
    def t_row(v):
        eu = np.concatenate([E[v], U[v]]).astype(np.float32)
        return eu @ w2  # [128]

    t1 = t_row(1)
    patch_even = np.empty((1, TROW), np.float32)
    patch_even[0, 0:F] = NEG                      # row 0: all -1e30
    patch_even[0, F : F + H] = NEG                # row 1 A-half
    patch_even[0, F + H : TROW] = t1[H:F]         # row 1 B-half = real
    patch_odd = np.empty((1, TROW), np.float32)
    patch_odd[0, 0:F] = t_row(HALF)
    patch_odd[0, F:TROW] = t_row(HALF + 1)

    p64 = np.zeros((P, H), dtype=bf16)
    p64[H:P, 0:H] = np.eye(H, dtype=bf16)

    owt = np.empty((H + 1, 2), np.float32)
    owt[0:H, :] = out_w.T
    owt[H, :] = out_b + out_w @ conv_b

    in_maps = []
    for c in range(NCORES):
        parity = c % 2
        k = c // 2
        half0 = 0 if parity == 0 else HALF
        # alpha slab: rows [half0, half0+ALPHA)
        a_cols = pack_cols(half0, ALPHA)
        eu_alpha = np.ascontiguousarray(EU_T[:, a_cols])
        # piece slab: rows [half0+ALPHA + k*NPIECE, +NPIECE)
        p_base = half0 + ALPHA + k * NPIECE
        p_cols = pack_cols(p_base, NPIECE)
        eu_piece = np.ascontiguousarray(EU_T[:, p_cols])

        # gather idx + parity masks
        sids = assign[:, c]                       # 32 sentence ids
        idx = np.zeros(C, np.int16)
        mvals = np.zeros(C, np.int8)
        for b in range(BS):
            sid = int(sids[b])
            st = int(csum[b])
            tb = int(T_b[sid])
            toks = sentences[0 : tb + 1, sid].astype(np.int64)  # positions 0..tb
            idx[st : st + tb + 1] = (toks >> 1).astype(np.int16)
            mvals[st : st + tb + 1] = (toks & 1).astype(np.int8)
            # rest of the slot (terminator and/or padding) stays idx 0, m 0
        wrapped = np.zeros((32, C // 16), np.int16)
        wr = idx.reshape(C // 16, 16).T           # [16, C/16]
        wrapped[0:16, :] = wr
        wrapped[16:32, :] = wr
        mv_full = np.broadcast_to(
            mvals.astype(np.int8)[None, :], (P, C)
        )
        in_maps.append(
            {
                "eu_alpha": eu_alpha,
                "eu_piece": eu_piece,
                "w2": w2.astype(bf16),
                "patch": patch_even if parity == 0 else patch_odd,
                "par": np.array([[parity]], np.int32),
                "idx_in": np.ascontiguousarray(wrapped),
                "mv_in": np.ascontiguousarray(mv_full),
                "p64_in": p64,
                "owt_in": owt,
            }
        )
    meta = {"assign": assign, "C": C}
    return nc, in_maps, meta


_CACHE = {}


def kernel(sentences, E, U, conv_w, conv_b, out_w, out_b):
    import hashlib

    h = hashlib.sha1()
    for a in (sentences, E, U, conv_w, conv_b, out_w, out_b):
        h.update(np.ascontiguousarray(a).tobytes())
    key = h.digest()
    if _CACHE.get("key") != key:
        nc, in_maps, meta = prepare(
            sentences, E, U, conv_w, conv_b, out_w, out_b
        )
        _CACHE.update(nc=nc, in_maps=in_maps, meta=meta, key=key)
    nc, in_maps, meta = _CACHE["nc"], _CACHE["in_maps"], _CACHE["meta"]
    res = run_bass_kernel_spmd(nc, in_maps, list(range(NCORES)))
    out = np.empty((B, 2), np.float32)
    assign = meta["assign"]
    for c in range(NCORES):
        sc = res.results[c]["scores"]
        out[assign[:, c]] = sc
    return out
